# revision 14
# baseline (speedup 1.0000x reference)
"""Trainium2 Bass kernel for nn_Attention_45148696216373.

8-core data-parallel over tokens (B*S = 131072 -> 16384/core). All ops are
per-token, so each core runs a fused per-token pipeline over 128-token tiles
(tokens on SBUF partitions), 8 tiles per group:

  PE   : proj = x @ [Q(256)|K(128)|V(128)] + bias (fp16 weights, 512-wide,
         exactly one PSUM bank; bias rides a K=1 ones-row matmul).
  ACT  : stage PSUM -> SBUF fp16 (proj, per 2 sub-tiles; y, per j).
  Pool : msum[d] = sum_h q[h,d] (fold tree) and km[k,d] = k0[k,d]*msum[d]
         (SBUF-only engine: GPSIMD cannot touch PSUM).
  DVE  : P[h,k,d] = q[h,d]*km[k,d]; attn[h,k] = sum_d P (fp16 fold tree);
         aot[h, d*2+k] = attn[h,k]*v[k,d]   (all fp16 2x-mode tensor ops).
  DMA  : batched XBAR transpose aot [t, h-block] -> aoT[h] = [(d,k), (u,t)].
  PE   : y_j[o; u,t] = sum_r Wo2_r^T @ aoT[2j+r]  (Wo folded, fp16, PSUM acc).
  DMA  : fp16 output in [o, (u,j,t)] layout; host re-transposes + casts f32.

vs the previous DVE-bound version this moves the whole Y-combine (73% of DVE
time) onto PE via a DMA transpose, folds Wo and all scalar constants into a
single fp16 weight set, and keeps the bias adds on PE. Every logical tensor
has its own tile-pool tag (rings are per-tag; shared tags serialize via slot
WAR). Groups are software-pipelined: back-stage work (y-matmul/stage/out) is
emitted with a mild priority penalty (offset=-90 ~ one group) so the tile
scheduler keeps the proj->stage->DVE chain hot and back work fills gaps; x is
prefetched one group ahead so the SP DMA queue never head-of-line blocks on
the aot transposes. Steady state is DVE-bound at 7728ns/group (15/16 groups
at exactly that cadence); modeled 172.5us vs 271.4us baseline; rel err ~1e-3
(fp16 inputs/weights/chain, f32 PSUM accumulation).

Math identity (per token; raw attn carries the 1/32 = 1/(H*sqrt(D)) into Wo2):
  q,k0,v = x@W+b blocks; msum = sum_h q_h
  attn_raw[h,k] = sum_d q[h,d]*k0[k,d]*msum[d]
  y[j, o] = sum_{r,k,d} attn_raw[2j+r,k] * v[k,d] * Wo[o, r*64+d] / 32
"""

import os

if os.environ.get("JAX_PLATFORMS", "").strip().lower() == "cpu":
    os.environ.pop("JAX_PLATFORMS")

import numpy as np

B, S, DIM = 16, 8192, 128
H, KV, D = 4, 2, 64
T = B * S                 # 131072 tokens
NCORES = 8
TPC = T // NCORES         # 16384 tokens per core
TT = 128                  # tokens per tile (partition dim)
GS = 8                    # tiles per group
NG = TPC // (TT * GS)     # 16 groups per core

NQ, NK, NV, NM = 256, 128, 128, 64
OQ, OK, OV = 0, 256, 384
NMAIN = 512               # [Q|K|V] fp16 block (exactly one PSUM f32 bank)

_COMPILED = None


def _fold_weights(Wq, bq, Wk, bk, Wv, bv, Wo):
    """W_main [128, 512] (lhsT-side), b_main [512], Wo2 [2, 128, 128]."""
    j = np.arange(NQ)
    Wq_p = Wq[j % H, j // H, :]
    bq_p = bq[j % H, j // H]
    jk = np.arange(NK)
    Wk_p = Wk[jk % KV, jk // KV, :]
    bk_p = bk[jk % KV, jk // KV]
    Wv_p = Wv[jk % KV, jk // KV, :]
    bv_p = bv[jk % KV, jk // KV]

    # V block, d-major k-minor: col = OV + d*2 + k  -> Wv_p[k*64+d]
    d = np.arange(D).repeat(2)
    k = np.tile(np.arange(KV), D)
    Wv_dk = Wv_p[k * D + d, :]
    bv_dk = bv_p[k * D + d]

    W_main = np.concatenate([Wq_p, Wk_p, Wv_dk], axis=0).astype(np.float32)
    b_main = np.concatenate([bq_p, bk_p, bv_dk]).astype(np.float32)

    Wo2 = np.zeros((2, 128, 128), dtype=np.float32)
    for r in range(2):
        Wo2[r, d * 2 + k, :] = (Wo[:, r * D + d].T / 32.0)
    return W_main.T.copy(), b_main, Wo2


def _build_program():
    import concourse.bass as bass
    import concourse.tile as tile
    from concourse import bacc, mybir

    f32 = mybir.dt.float32
    f32r = mybir.dt.float32r
    bf16 = mybir.dt.bfloat16
    f16 = mybir.dt.float16

    nc = bacc.Bacc(
        "TRN2",
        target_bir_lowering=False,
        debug=False,
        enable_asserts=False,
        num_devices=NCORES,
    )

    xT_d = nc.dram_tensor("xT", [DIM, TPC], f16, kind="ExternalInput").ap()
    w_d = nc.dram_tensor("wmain", [DIM, NMAIN], f16, kind="ExternalInput").ap()
    b_d = nc.dram_tensor("bmain", [1, NMAIN], f16, kind="ExternalInput").ap()
    one_d = nc.dram_tensor("ones", [1, TT], f16, kind="ExternalInput").ap()
    wo2_d = nc.dram_tensor("wo2", [DIM, 256], f16, kind="ExternalInput").ap()
    y_d = nc.dram_tensor("y", [DIM, 2 * TPC], f16, kind="ExternalOutput").ap()

    with tile.TileContext(nc) as tc:
        with (
            tc.tile_pool(name="const", bufs=1) as cpool,
            tc.tile_pool(name="psum", bufs=1, space="PSUM") as ppool,
            tc.tile_pool(name="sbuf", bufs=1) as pool,
        ):
            w_sb = cpool.tile([DIM, NMAIN], f16, tag="w")
            nc.sync.dma_start(w_sb[:], w_d[:, :])
            b_sb = cpool.tile([1, NMAIN], f16, tag="b")
            nc.sync.dma_start(b_sb[:], b_d[:, :])
            one_sb = cpool.tile([1, TT], f16, tag="one")
            nc.sync.dma_start(one_sb[:], one_d[:, :])
            wo2_sb = cpool.tile([DIM, 256], f16, tag="wo2")
            nc.sync.dma_start(wo2_sb[:], wo2_d[:, :])

            aoT_ring = {}
            xt_ring = {}

            def fetch_x(g):
                xt = pool.tile([DIM, GS * TT], f16, name="xt", tag="xt", bufs=8)
                nc.sync.dma_start(xt[:], xT_d[:, g * GS * TT:(g + 1) * GS * TT])
                xt_ring[g] = xt

            def front(g):
                if g + 1 < NG:
                    fetch_x(g + 1)   # prefetch: dispatches before transp(g) waits
                xt = xt_ring.pop(g)

                stg = pool.tile([TT, GS, NMAIN], f16, name="stg", tag="stg", bufs=6)
                for s in range(0, GS, 2):
                    pm = ppool.tile([TT, 2, NMAIN], f32, name="pm", tag="pm", bufs=2)
                    for v in range(2):
                        u = s + v
                        xu = xt[:, u * TT:(u + 1) * TT]
                        nc.tensor.matmul(out=pm[:, v, :], lhsT=one_sb[:, :],
                                         rhs=b_sb[:, :], start=True, stop=False)
                        nc.tensor.matmul(out=pm[:, v, :], lhsT=xu,
                                         rhs=w_sb[:, :], start=False, stop=True)
                    nc.scalar.copy(stg[:, s:s + 2, :], pm[:])

                # msum[u,d] = sum_h q[u,h,d] on Pool (SBUF-only engine), then
                # km[u,k,d] = k0[u,k,d] * msum[u,d]; per half-group so DVE's
                # P ops can start after the first 4 stagings
                m1 = pool.tile([TT, GS, NM], f16, name="m1", tag="m1", bufs=2)
                m2 = pool.tile([TT, GS, NM], f16, name="m2", tag="m2", bufs=2)
                msum = pool.tile([TT, GS, NM], f16, name="msum", tag="msum", bufs=2)
                km = pool.tile([TT, GS, KV, D], f16, name="km", tag="km", bufs=4)
                for hf in range(2):
                    hs = slice(hf * 4, (hf + 1) * 4)
                    nc.gpsimd.tensor_add(m1[:, hs], stg[:, hs, 0:64],
                                         stg[:, hs, 64:128])
                    nc.gpsimd.tensor_add(m2[:, hs], stg[:, hs, 128:192],
                                         stg[:, hs, 192:256])
                    nc.gpsimd.tensor_add(msum[:, hs], m1[:, hs], m2[:, hs])
                    nc.gpsimd.tensor_mul(
                        km[:, hs],
                        stg[:, hs, OK:OK + NK]
                        .rearrange("p u (k d) -> p u k d", k=KV),
                        msum[:, hs].unsqueeze(2).broadcast_to([TT, 4, KV, D]),
                    )

                # P[u,h,k,d] = q[u,h,d] * km[u,k,d]   (DVE fp16 2x, per-2u)
                P = pool.tile([TT, GS, H * KV, D], f16, name="P", tag="P", bufs=4)
                for s in range(0, GS, 2):
                    nc.vector.tensor_mul(
                        P[:, s:s + 2]
                        .rearrange("p u (h k) d -> p u h k d", k=KV),
                        stg[:, s:s + 2, OQ:OQ + NQ]
                        .rearrange("p u (h d) -> p u h d", h=H)
                        .unsqueeze(3).broadcast_to([TT, 2, H, KV, D]),
                        km[:, s:s + 2].unsqueeze(2)
                        .broadcast_to([TT, 2, H, KV, D]),
                    )

                # attn[u,(h,k)] = sum_d P : fp16 fold tree (2x except last)
                A1 = pool.tile([TT, GS, 8, 32], f16, name="A1", tag="A1", bufs=3)
                nc.vector.tensor_add(A1[:], P[:, :, :, 0:32], P[:, :, :, 32:64])
                A2 = pool.tile([TT, GS, 8, 16], f16, name="A2", tag="A2", bufs=3)
                nc.vector.tensor_add(A2[:], A1[:, :, :, 0:16], A1[:, :, :, 16:32])
                A3 = pool.tile([TT, GS, 8, 8], f16, name="A3", tag="A3", bufs=3)
                nc.vector.tensor_add(A3[:], A2[:, :, :, 0:8], A2[:, :, :, 8:16])
                A4 = pool.tile([TT, GS, 8, 4], f16, name="A4", tag="A4", bufs=3)
                nc.vector.tensor_add(A4[:], A3[:, :, :, 0:4], A3[:, :, :, 4:8])
                A5 = pool.tile([TT, GS, 8, 2], f16, name="A5", tag="A5", bufs=3)
                nc.vector.tensor_add(A5[:], A4[:, :, :, 0:2], A4[:, :, :, 2:4])
                attn = pool.tile([TT, GS, 8], f16, name="attn", tag="attn", bufs=3)
                nc.vector.tensor_add(attn[:], A5[:, :, :, 0], A5[:, :, :, 1])

                # aot[t, h, u, d*2+k] = attn[u,h,k] * v[u,k,d]  (DVE 2x, per-2u)
                aot = pool.tile([TT, H, GS, NV], f16, name="aot", tag="aot", bufs=4)
                for s in range(0, GS, 2):
                    nc.vector.tensor_mul(
                        aot[:, :, s:s + 2, :]
                        .rearrange("p h u (d k) -> p h u d k", k=KV),
                        attn[:, s:s + 2]
                        .rearrange("p u (h k) -> p h u k", k=KV)
                        .unsqueeze(3).broadcast_to([TT, H, 2, D, KV]),
                        stg[:, s:s + 2, OV:OV + NV]
                        .rearrange("p u (d k) -> p u d k", k=KV)
                        .unsqueeze(1).broadcast_to([TT, H, 2, D, KV]),
                    )

                # DMA XBAR transpose per h-pair: [t, (h,u,dk)] -> [(dk), h, u, t]
                aoT = []
                for hp in range(2):
                    th = pool.tile([NV, 2, GS, TT], f16, name=f"aoT{hp}",
                                   tag=f"aoT{hp}", bufs=3)
                    nc.sync.dma_start_transpose(
                        th[:], aot[:, 2 * hp:2 * hp + 2]
                        .rearrange("p h u f -> p (h u f)"))
                    aoT.append(th)
                aoT_ring[g] = aoT

            def back(g):
                aoT = aoT_ring.pop(g)
                # y_j[o; (u,t)] = sum_r Wo2_r^T @ aoT[2j+r]  (PE, fp16)
                y_sb = pool.tile([DIM, GS, 2, TT], f16, name="ysb", tag="ysb", bufs=3)
                for j in range(2):
                    yp = ppool.tile([DIM, GS * TT], f32, name="yp", tag="yp", bufs=2)
                    for half in range(2):
                        for r in range(2):
                            nc.tensor.matmul(
                                out=yp[:, half * 4 * TT:(half + 1) * 4 * TT],
                                lhsT=wo2_sb[:, r * 128:(r + 1) * 128],
                                rhs=aoT[j][:, r, half * 4:(half + 1) * 4]
                                .rearrange("p u t -> p (u t)"),
                                start=(r == 0), stop=(r == 1),
                            )
                    nc.scalar.copy(
                        y_sb[:, :, j, :],
                        yp[:].rearrange("p (u t) -> p u t", t=TT),
                    )

                nc.sync.dma_start(
                    y_d[:, g * GS * 2 * TT:(g + 1) * GS * 2 * TT],
                    y_sb[:].rearrange("p u j t -> p (u j t)"),
                )

            SKEW = 4
            fetch_x(0)
            for i in range(NG + SKEW):
                if i >= SKEW:
                    back(i - SKEW)
                if i < NG:
                    front(i)

    nc.compile()
    return nc


def kernel(x, Wq, bq, Wk, bk, Wv, bv, Wo):
    global _COMPILED
    from concourse.bass_utils import run_bass_kernel_spmd

    x = np.asarray(x, dtype=np.float32)
    W_main, b_main, Wo2 = _fold_weights(
        np.asarray(Wq, np.float32), np.asarray(bq, np.float32),
        np.asarray(Wk, np.float32), np.asarray(bk, np.float32),
        np.asarray(Wv, np.float32), np.asarray(bv, np.float32),
        np.asarray(Wo, np.float32),
    )

    if _COMPILED is None:
        _COMPILED = _build_program()
    nc = _COMPILED

    x2d = x.reshape(T, DIM)
    ones = np.ones((1, TT), dtype=np.float32)
    wo2 = np.concatenate([Wo2[0], Wo2[1]], axis=1).astype(np.float16)

    in_maps = []
    for c in range(NCORES):
        shard = x2d[c * TPC:(c + 1) * TPC]
        in_maps.append({
            "xT": np.ascontiguousarray(shard.T).astype(np.float16),
            "wmain": W_main.astype(np.float16),
            "bmain": b_main.reshape(1, NMAIN).astype(np.float16),
            "ones": ones.astype(np.float16),
            "wo2": wo2,
        })

    res = run_bass_kernel_spmd(nc, in_maps, list(range(NCORES)))
    outs = []
    for c in range(NCORES):
        a = np.asarray(res.results[c]["y"])          # [128, 2*TPC] fp16
        a = a.T.reshape(NG, GS, 2, TT, DIM)          # [g, u, j, t, o]
        a = a.transpose(0, 1, 3, 2, 4).reshape(2 * TPC, DIM)
        outs.append(a)
    Y = np.concatenate(outs, axis=0).astype(np.float32)
    return Y.reshape(B, 2 * S, DIM)

